# revision 1
# baseline (speedup 1.0000x reference)
"""Causal self-attention (B=4, S=2048, D=1024, H=16) on 8 trn2 cores.

Sharding: core c -> (batch b = c//2, head-half hh = c%2). Each core:
  - computes Q/K/V projections for its batch restricted to its 8 heads
    (512 of the 1024 feature columns),
  - runs causal attention for those heads,
  - computes a partial out-projection part = attnO @ w_o[rows of its heads].
Host: out[b] = part[2b] + part[2b+1] + (b_v @ w_o + b_o).
(The V bias contributes b_v @ w_o to the output because softmax rows sum
to 1; the out-proj bias is b_o. Both are token-independent row vectors.)

On-core layouts (feature-major where it kills transposes):
  xT   [1024,2048]  x transposed via PE transpose (8 tiles [128,2048])
  qt/kt[512 ,2048]  feature-major; tile g holds heads 2g,2g+1
  v_aug[2048, 520]  token-major, 65 cols/head: 64 V cols + a ones column
                    (the ones column makes the PV matmul also produce the
                    softmax denominator as PSUM row 64)
  scores ST [nk,mq] per 128-row tile; exp on ACT (scale=1/8, no max
                    subtraction -- scores are ~N(0,1), exp is safe in fp32)
  causal mask       affine_select (GPSIMD) zeroes exp(masked) entries;
                    fully-masked column prefixes of diagonal tiles are
                    skipped in the score/exp/PV ops entirely
  normalization     reciprocal of sum row + PE ones-broadcast + DVE mul
Projections, scores and the out-projection run in float32r (full PE rate
at N>=512 moving, ~1e-4 rounding); the exp(scores) tiles and V are bf16
(the P@V stage), which dominates the ~2e-3 end-to-end relative error.
"""

import sys

if "/opt/trn_rl_repo" not in sys.path:
    sys.path.insert(0, "/opt/trn_rl_repo")

import numpy as np

import concourse.bass as bass
import concourse.tile as tile
from concourse import bacc, mybir
from concourse.bass_utils import run_bass_kernel_spmd
from concourse.masks import make_identity

N_CORES = 8
S = 2048
D = 1024
DH = 512          # per-core feature width (8 heads x 64)
HD = 64           # head dim
NH_LOC = 8        # heads per core
F32 = mybir.dt.float32
F32R = mybir.dt.float32r
EXP = mybir.ActivationFunctionType.Exp
GE = mybir.AluOpType.is_ge

_PROGRAM = None


def _build_program(n_repeat=1):
    nc = bacc.Bacc("TRN2", target_bir_lowering=False, debug=False,
                   num_devices=N_CORES)
    x_d = nc.dram_tensor("x", [S, D], F32, kind="ExternalInput").ap()
    wq_d = nc.dram_tensor("wq", [D, DH], F32, kind="ExternalInput").ap()
    wk_d = nc.dram_tensor("wk", [D, DH], F32, kind="ExternalInput").ap()
    wv_d = nc.dram_tensor("wv", [D, DH], F32, kind="ExternalInput").ap()
    wo_d = nc.dram_tensor("wo", [DH, D], F32, kind="ExternalInput").ap()
    bq_d = nc.dram_tensor("bq", [DH], F32, kind="ExternalInput").ap()
    bk_d = nc.dram_tensor("bk", [DH], F32, kind="ExternalInput").ap()
    part_d = nc.dram_tensor("part", [S, D], F32, kind="ExternalOutput").ap()

    with tile.TileContext(nc) as tc:
        for _ in range(n_repeat):
            _emit(nc, tc, x_d, wq_d, wk_d, wv_d, wo_d, bq_d, bk_d, part_d)
    nc.compile()
    return nc


def _emit(nc, tc, x_d, wq_d, wk_d, wv_d, wo_d, bq_d, bk_d, part_d):
    """Emission is hand-pipelined: per-engine instruction order follows
    emission order, so work is zipped so the PE always has filler matmuls
    queued behind attention iterations that pace on the ACT engine:

      [transpose x (PE) || V projection || wv/wq/wk DMAs]
      [Q0/K0 projection]
      [attention pair0 || Q1/K1 projection]   (4 attn iters : 1 proj unit)
      [attention pair1 || Q2/K2 projection]
      [attention pair2 || Q3/K3 projection]
      [attention pair3 || per-chunk out-projection]

    Q/K tiles rotate through 2 slots per tag (pair g is dead once its
    attention is done), which is what makes everything fit in SBUF.
    """
    from contextlib import ExitStack

    BF16 = mybir.dt.bfloat16
    es = ExitStack()
    with es:
        const_pool = es.enter_context(tc.tile_pool(name="const", bufs=1))
        v_pool = es.enter_context(tc.tile_pool(name="vaug", bufs=1))
        qkv_pool = es.enter_context(tc.tile_pool(name="qkv", bufs=2))
        exp_pool = es.enter_context(tc.tile_pool(name="ex", bufs=6))
        rc_pool = es.enter_context(tc.tile_pool(name="rc", bufs=3))
        pp_pool = es.enter_context(
            tc.tile_pool(name="pp", bufs=2, space="PSUM"))

        ident = const_pool.tile([128, 128], F32, name="ident", tag="ident")
        make_identity(nc, ident)
        ones_bf = const_pool.tile([128, NH_LOC], BF16, name="ones_bf",
                                  tag="ones")
        nc.vector.memset(ones_bf, 1.0)
        ones_f32 = const_pool.tile([128, HD], F32, name="ones_f32",
                                   tag="onesf")
        nc.vector.memset(ones_f32, 1.0)
        # f32r view for the normalization broadcast matmul (f32r tiles
        # cannot be memset directly; a convert-copy is the legal producer).
        ones_fr = const_pool.tile([128, HD], F32R, name="ones_fr",
                                  tag="onesfr")
        nc.vector.tensor_copy(ones_fr[:], ones_f32[:])
        bq_sb = const_pool.tile([128, 4], F32, name="bq_sb", tag="bq")
        bk_sb = const_pool.tile([128, 4], F32, name="bk_sb", tag="bk")

        v_aug = [v_pool.tile([128, NH_LOC * (HD + 1)], BF16,
                             name=f"va{t}", tag=f"va{t}")
                 for t in range(16)]

        xt_cm = tc.tile_pool(name="xtp", bufs=1, side="right")
        xt_pool = xt_cm.__enter__()
        xt = [xt_pool.tile([128, S], F32R, name=f"xt{k}", tag=f"xt{k}")
              for k in range(8)]

        # ---- Phase A+V: transpose x, V projection zipped in ----------
        with (
            tc.tile_pool(name="xs", bufs=8, side="right") as xs_pool,
            tc.tile_pool(name="tp", bufs=4, space="PSUM") as tp_pool,
            tc.tile_pool(name="wv", bufs=8, side="right") as wv_pool,
        ):
            xs_tiles = {}

            def load_x(mg, split_first=False):
                xs = []
                for i in range(4):
                    xst = xs_pool.tile([128, D], F32,
                                       name=f"xs{mg}_{i}", tag="xs")
                    rows = slice((4 * mg + i) * 128, (4 * mg + i + 1) * 128)
                    if split_first and i == 0:
                        # split the very first tile so the first transpose
                        # can start after ~64KB instead of 512KB
                        for q in range(4):
                            cs = slice(q * 256, (q + 1) * 256)
                            nc.sync.dma_start(out=xst[:, cs],
                                              in_=x_d[rows, cs])
                    else:
                        nc.sync.dma_start(out=xst, in_=x_d[rows, :])
                    xs.append(xst)
                xs_tiles[mg] = xs

            # x for the first two token groups goes first so the PE isn't
            # stuck behind the wv transfer at kernel start.
            load_x(0, split_first=True)
            load_x(1)
            wtv = []
            for kc in range(8):
                w_t = wv_pool.tile([128, DH], F32R, name=f"wv{kc}", tag="wv")
                nc.sync.dma_start(
                    out=w_t,
                    in_=wv_d[kc * 128:(kc + 1) * 128, :].bitcast(F32R))
                wtv.append(w_t)
            # bias rows are tiny strided DMAs; keep them behind the bulk
            # transfers so they don't delay the first transposes
            for g in range(4):
                sl = slice(g * 128, (g + 1) * 128)
                nc.sync.dma_start(
                    out=bq_sb[:, g:g + 1],
                    in_=bq_d[sl].rearrange("(p one) -> p one", one=1))
                nc.sync.dma_start(
                    out=bk_sb[:, g:g + 1],
                    in_=bk_d[sl].rearrange("(p one) -> p one", one=1))
            for mg in range(4):
                if mg + 2 < 4:
                    load_x(mg + 2)
                xs = xs_tiles.pop(mg)
                if mg == 0:
                    # First group: transpose per x tile so the PE starts as
                    # soon as the first 512KB lands instead of waiting for
                    # all four tiles.
                    for i in range(4):
                        for kc in range(8):
                            tp = tp_pool.tile([128, 512], F32,
                                              name=f"tp0_{i}_{kc}", tag="tp")
                            nc.tensor.transpose(
                                tp[:, 0:128],
                                xs[i][:, kc * 128:(kc + 1) * 128],
                                ident[:])
                            nc.vector.tensor_copy(
                                xt[kc][:, i * 128:(i + 1) * 128],
                                tp[:, 0:128])
                else:
                    for kc in range(8):
                        tp = tp_pool.tile([128, 512], F32,
                                          name=f"tp{mg}_{kc}", tag="tp")
                        for i in range(4):
                            nc.tensor.transpose(
                                tp[:, i * 128:(i + 1) * 128],
                                xs[i][:, kc * 128:(kc + 1) * 128],
                                ident[:])
                        nc.vector.tensor_copy(
                            xt[kc][:, mg * 512:(mg + 1) * 512], tp[:])
                # V projection for this token group (PE filler while the
                # next group's x tiles stream in).
                for mt in range(4 * mg, 4 * mg + 4):
                    pp = pp_pool.tile([128, 512], F32,
                                      name=f"ppv{mt}", tag="pp")
                    for kc in range(8):
                        nc.tensor.matmul(
                            pp[:],
                            xt[kc][:, mt * 128:(mt + 1) * 128],
                            wtv[kc][:],
                            start=(kc == 0), stop=(kc == 7))
                    va3 = v_aug[mt].rearrange("p (h c) -> p h c", h=NH_LOC)
                    nc.vector.tensor_copy(
                        va3[:, :, 0:HD],
                        pp[:].rearrange("p (h c) -> p h c", h=NH_LOC))
                    nc.vector.tensor_copy(
                        va3[:, :, HD:HD + 1],
                        ones_bf[:, 0:NH_LOC].rearrange(
                            "p (h one) -> p h one", one=1))

        # ---- attention-phase pools (open after tp frees its banks) ----
        w_cm = tc.tile_pool(name="wp", bufs=32, side="right")
        w_pool = w_cm.__enter__()
        st_pool = es.enter_context(
            tc.tile_pool(name="st", bufs=2, space="PSUM"))
        otp_pool = es.enter_context(
            tc.tile_pool(name="ops", bufs=2, space="PSUM"))
        ot_pool = es.enter_context(tc.tile_pool(name="otl", bufs=16))

        qt = {}
        kt = {}
        ot_map = {}

        def emit_proj_units(g):
            """Q/K projection for pair g as a list of emission closures."""
            units = []

            def load(wd, pname):
                wt = []
                for kc in range(8):
                    w_t = w_pool.tile([128, 128], F32R,
                                      name=f"w{pname}{g}_{kc}", tag="w")
                    nc.sync.dma_start(
                        out=w_t,
                        in_=wd[kc * 128:(kc + 1) * 128,
                               g * 128:(g + 1) * 128].bitcast(F32R))
                    wt.append(w_t)
                return wt

            def alloc_out(which):
                t = qkv_pool.tile([128, S], F32R,
                                  name=f"{which}t{g}", tag=which)
                (qt if which == "qt" else kt)[g] = t
                return t

            state = {}

            def u_load_q():
                state["wq"] = load(wq_d, "q")
                state["qt"] = alloc_out("qt")

            def u_load_k():
                state["wk"] = load(wk_d, "k")
                state["kt"] = alloc_out("kt")

            units.append(u_load_q)
            units.append(u_load_k)

            def mk_mc(which, bias_sb, mc):
                def u():
                    wt = state["wq" if which == "qt" else "wk"]
                    out_t = state[which[:2]]
                    pp = pp_pool.tile([128, 512], F32,
                                      name=f"pp{which}{g}_{mc}", tag="pp")
                    for kc in range(8):
                        nc.tensor.matmul(
                            pp[:],
                            wt[kc][:],
                            xt[kc][:, mc * 512:(mc + 1) * 512],
                            start=(kc == 0), stop=(kc == 7))
                    nc.vector.tensor_scalar_add(
                        out_t[:, mc * 512:(mc + 1) * 512],
                        pp[:], bias_sb[:, g:g + 1])
                return u

            for mc in range(4):
                units.append(mk_mc("qt", bq_sb, mc))
            for mc in range(4):
                units.append(mk_mc("kt", bk_sb, mc))
            return units

        def emit_att_iter(g, j, t, t_max):
            # Columns [0, z) of a diagonal tile are fully masked: skip them
            # in the score matmul, exp, mask, and PV accumulation entirely
            # (the PV start=True tile always covers the full width, so the
            # untouched PSUM columns keep their accumulated values).
            d = t - 4 * j
            z = 0 if d < 0 else 128 * d
            w = 512 - z
            mq = slice(j * 512 + z, (j + 1) * 512)
            nk = slice(t * 128, (t + 1) * 128)
            st = st_pool.tile([128, 1024], F32,
                              name=f"st{j}_{g}_{t}", tag="st")
            for hl in range(2):
                dsl = slice(hl * 64, hl * 64 + 64)
                nc.tensor.matmul(
                    st[:, hl * 512:hl * 512 + w],
                    kt[g][dsl, nk], qt[g][dsl, mq],
                    start=True, stop=True)
            ex = exp_pool.tile([128, 1024], BF16,
                               name=f"ex{j}_{g}_{t}", tag="ex")
            if d < 0:
                # off-diagonal: both head halves in one wide ACT op
                nc.scalar.activation(ex[:, 0:1024], st[:, 0:1024],
                                     EXP, scale=0.125)
            else:
                # diagonal: both head halves in one strided 3D op each for
                # exp and mask (the head dim is a stride-512 middle axis;
                # the mask predicate ignores it via a 0-step pattern pair)
                st3 = st.rearrange("p (h q) -> p h q", h=2)[:, :, 0:w]
                ex3 = ex.rearrange("p (h q) -> p h q", h=2)[:, :, 0:w]
                nc.scalar.activation(ex3, st3, EXP, scale=0.125)
                # keep where local_mq >= local_nk: y - p >= 0
                nc.gpsimd.affine_select(
                    out=ex3, in_=ex3,
                    compare_op=GE, fill=0.0, base=0,
                    channel_multiplier=-1,
                    pattern=[[0, 2], [1, w]])
            o_ps = ot_map[("ps", g, j)]
            for hl in range(2):
                h = 2 * g + hl
                nc.tensor.matmul(
                    o_ps[hl][:, z:512],
                    v_aug[t][:, 65 * h:65 * h + 65],
                    ex[:, hl * 512:hl * 512 + w],
                    start=(t == 0), stop=(t == t_max - 1))

        def emit_normalize(g, j):
            o_ps = ot_map.pop(("ps", g, j))
            ot_t = ot_pool.tile([128, 512], F32R,
                                name=f"ot{j}_{g}", tag="ot")
            ot_map[(j, g)] = ot_t
            for hl in range(2):
                # Copy PSUM out early to free the accumulation bank, then
                # 1/sum on the sum row, broadcast it across partitions with
                # a rank-1 PE matmul (ones^T @ recip), and scale.
                ocp = rc_pool.tile([65, 512], F32R,
                                   name=f"ocp{j}_{g}_{hl}", tag="ocp")
                nc.vector.tensor_copy(ocp[:], o_ps[hl][:])
                with nc.allow_low_precision(reason="f32r recip row"):
                    nc.vector.reciprocal(ocp[64:65, :], ocp[64:65, :])
                bc = pp_pool.tile([64, 512], F32,
                                  name=f"bc{j}_{g}_{hl}", tag="pp")
                nc.tensor.matmul(bc[:], ones_fr[64:65, 0:HD],
                                 ocp[64:65, :], start=True, stop=True)
                nc.vector.tensor_mul(
                    ot_t[64 * hl:64 * hl + 64, :],
                    ocp[0:64, :], bc[:])

        def att_iters_for_pair(g):
            iters = []
            for j in range(4):
                t_max = 4 * (j + 1)
                def mk_alloc(g=g, j=j):
                    def u():
                        ot_map[("ps", g, j)] = [
                            otp_pool.tile([65, 512], F32,
                                          name=f"o{j}_{g}_{hl}", tag="ops")
                            for hl in range(2)]
                    return u
                iters.append(mk_alloc())
                for t in range(t_max):
                    def mk(g=g, j=j, t=t, t_max=t_max):
                        def u():
                            emit_att_iter(g, j, t, t_max)
                        return u
                    iters.append(mk())
                def mk_norm(g=g, j=j):
                    def u():
                        emit_normalize(g, j)
                    return u
                iters.append(mk_norm())
            return iters

        def zip_emit(primary, filler):
            n_p, n_f = len(primary), len(filler)
            fi = 0
            for i, p in enumerate(primary):
                p()
                while fi < n_f and (i + 1) * n_f >= (fi + 1) * n_p:
                    filler[fi]()
                    fi += 1
            while fi < n_f:
                filler[fi]()
                fi += 1

        # Q0/K0 first, then attention(g) zipped with projections(g+1).
        for u in emit_proj_units(0):
            u()
        for g in range(3):
            zip_emit(att_iters_for_pair(g), emit_proj_units(g + 1))

        # xT and the projection weights are dead now; free them (they live
        # on the right-side SBUF stack, popped LIFO: wp then xtp) before the
        # out-projection pools open so the SBUF budget holds.
        w_cm.__exit__(None, None, None)
        xt_cm.__exit__(None, None, None)

        wo_pool = es.enter_context(tc.tile_pool(name="wo", bufs=4))
        os_pool = es.enter_context(tc.tile_pool(name="os", bufs=4))
        wo_t = []
        for fc in range(4):
            w_t = wo_pool.tile([128, D], F32R, name=f"wo{fc}", tag=f"wo{fc}")
            nc.sync.dma_start(
                out=w_t,
                in_=wo_d[fc * 128:(fc + 1) * 128, :].bitcast(F32R))
            wo_t.append(w_t)

        def emit_outproj_one(j, mt, nck):
            msl = slice((mt - 4 * j) * 128, (mt - 4 * j) * 128 + 128)
            op = pp_pool.tile([128, 512], F32,
                              name=f"op{mt}_{nck}", tag="pp")
            for g in range(4):
                nc.tensor.matmul(
                    op[:],
                    ot_map[(j, g)][:, msl],
                    wo_t[g][:, nck * 512:(nck + 1) * 512],
                    start=(g == 0), stop=(g == 3))
            osb = os_pool.tile([128, 512], F32,
                               name=f"os{mt}_{nck}", tag="os")
            nc.vector.tensor_copy(osb[:], op[:])
            nc.sync.dma_start(
                out=part_d[mt * 128:(mt + 1) * 128,
                           nck * 512:(nck + 1) * 512],
                in_=osb[:])

        def outproj_units(j):
            units = []
            for mt in range(4 * j, 4 * j + 4):
                for nck in range(2):
                    def u(j=j, mt=mt, nck=nck):
                        emit_outproj_one(j, mt, nck)
                    units.append(u)
            return units

        # Last pair: chunk j-1's out-projection is zipped between chunk j's
        # attention iterations so the PE keeps filler work while the
        # normalize chain drains; chunk 3's out-projection closes the tail.
        g = 3
        for j in range(4):
            t_max = 4 * (j + 1)

            def mk_alloc(j=j):
                def u():
                    ot_map[("ps", g, j)] = [
                        otp_pool.tile([65, 512], F32,
                                      name=f"o{j}_{g}_{hl}", tag="ops")
                        for hl in range(2)]
                return u
            units = [mk_alloc()]
            for t in range(t_max):
                def mk(j=j, t=t, t_max=t_max):
                    def u():
                        emit_att_iter(g, j, t, t_max)
                    return u
                units.append(mk())

            def mk_norm(j=j):
                def u():
                    emit_normalize(g, j)
                return u
            units.append(mk_norm())
            zip_emit(units, outproj_units(j - 1) if j > 0 else [])
        for u in outproj_units(3):
            u()


def _get_program():
    global _PROGRAM
    if _PROGRAM is None:
        _PROGRAM = _build_program()
    return _PROGRAM


_EXEC = None


def _get_executor():
    """Build the sharded PJRT executable once and reuse it across calls.

    Mirrors bass2jax.run_bass_via_pjrt's multi-core branch, but caches the
    jitted callable so repeat kernel() calls skip retracing/recompilation.
    Returns (fn, in_names, out_names, out_shapes). fn takes globally
    concatenated inputs (n_cores*dim0, ...) plus donated zero output
    buffers, and returns concatenated outputs.
    """
    global _EXEC
    if _EXEC is None:
        import jax
        from jax.experimental.shard_map import shard_map
        from jax.sharding import Mesh, PartitionSpec

        from concourse import bass2jax

        bass2jax.install_neuronx_cc_hook()
        nc = _get_program()
        part_name = (nc.partition_id_tensor.name
                     if nc.partition_id_tensor else None)
        in_names, out_names, out_avals = [], [], []
        for alloc in nc.m.functions[0].allocations:
            if not isinstance(alloc, mybir.MemoryLocationSet):
                continue
            name = alloc.memorylocations[0].name
            if alloc.kind == "ExternalInput":
                if name != part_name:
                    in_names.append(name)
            elif alloc.kind == "ExternalOutput":
                out_names.append(name)
                out_avals.append(jax.core.ShapedArray(
                    tuple(alloc.tensor_shape), mybir.dt.np(alloc.dtype)))
        n_params = len(in_names)
        all_in = tuple(in_names) + tuple(out_names)
        if part_name is not None:
            all_in = all_in + (part_name,)

        def _body(*args):
            operands = list(args)
            if part_name is not None:
                operands.append(bass2jax.partition_id_tensor())
            outs = bass2jax._bass_exec_p.bind(
                *operands,
                out_avals=tuple(out_avals),
                in_names=all_in,
                out_names=tuple(out_names),
                lowering_input_output_aliases=(),
                sim_require_finite=True,
                sim_require_nnan=True,
                nc=nc)
            return tuple(outs)

        devices = jax.devices()[:N_CORES]
        mesh = Mesh(np.asarray(devices), ("core",))
        n_bufs = n_params + len(out_names)
        mapped = shard_map(_body, mesh=mesh,
                           in_specs=(PartitionSpec("core"),) * n_bufs,
                           out_specs=(PartitionSpec("core"),) * len(out_names),
                           check_rep=False)
        fn = jax.jit(mapped,
                     donate_argnums=tuple(range(n_params, n_bufs)),
                     keep_unused=True)
        # Non-donating twin: lets a timing loop reuse device-resident
        # argument buffers across calls (we write every element of every
        # output, so uninitialized result buffers are fine).
        fn_nodonate = jax.jit(mapped, keep_unused=True)
        out_shapes = [tuple(a.shape) for a in out_avals]
        _EXEC = (fn, fn_nodonate, in_names, out_names, out_shapes, mesh)
    return _EXEC


def run_cores(in_maps):
    """Run the SPMD program on 8 cores via the cached executable."""
    fn, _, in_names, out_names, out_shapes = _get_executor()[:5]
    concat_in = [np.concatenate([in_maps[c][n] for c in range(N_CORES)],
                                axis=0) for n in in_names]
    zeros = [np.zeros((N_CORES * s[0],) + s[1:], np.float32)
             for s in out_shapes]
    outs = fn(*concat_in, *zeros)
    res = []
    for c in range(N_CORES):
        res.append({
            n: np.asarray(outs[i]).reshape((N_CORES,) + out_shapes[i])[c]
            for i, n in enumerate(out_names)})
    return res


def make_in_maps(x, w_q, b_q, w_k, b_k, w_v, b_v, w_o, b_o):
    in_maps = []
    for c in range(N_CORES):
        b, hh = divmod(c, 2)
        cols = slice(hh * DH, (hh + 1) * DH)
        in_maps.append({
            "x": np.ascontiguousarray(x[b]),
            "wq": np.ascontiguousarray(w_q[:, cols]),
            "wk": np.ascontiguousarray(w_k[:, cols]),
            "wv": np.ascontiguousarray(w_v[:, cols]),
            "wo": np.ascontiguousarray(w_o[cols, :]),
            "bq": np.ascontiguousarray(b_q[cols]),
            "bk": np.ascontiguousarray(b_k[cols]),
        })
    return in_maps


def combine(parts, b_v, w_o, b_o):
    corr = (b_v @ w_o + b_o).astype(np.float32)
    out = np.empty((4, S, D), dtype=np.float32)
    for b in range(4):
        out[b] = parts[2 * b] + parts[2 * b + 1] + corr
    return out


def kernel(x, w_q, b_q, w_k, b_k, w_v, b_v, w_o, b_o):
    x = np.asarray(x, dtype=np.float32)
    w_q = np.asarray(w_q, dtype=np.float32)
    b_q = np.asarray(b_q, dtype=np.float32)
    w_k = np.asarray(w_k, dtype=np.float32)
    b_k = np.asarray(b_k, dtype=np.float32)
    w_v = np.asarray(w_v, dtype=np.float32)
    b_v = np.asarray(b_v, dtype=np.float32)
    w_o = np.asarray(w_o, dtype=np.float32)
    b_o = np.asarray(b_o, dtype=np.float32)

    in_maps = make_in_maps(x, w_q, b_q, w_k, b_k, w_v, b_v, w_o, b_o)
    res = run_cores(in_maps)
    parts = [res[c]["part"] for c in range(N_CORES)]
    return combine(parts, b_v, w_o, b_o)



# revision 2
# speedup vs baseline: 1.1092x; 1.1092x over previous
"""Causal self-attention (B=4, S=2048, D=1024, H=16) on 8 trn2 cores.

Sharding: core c -> (batch b = c//2, head-half hh = c%2). Each core:
  - computes Q/K/V projections for its batch restricted to its 8 heads
    (512 of the 1024 feature columns),
  - runs causal attention for those heads,
  - computes a partial out-projection part = attnO @ w_o[rows of its heads].
Host: out[b] = part[2b] + part[2b+1] + (b_v @ w_o + b_o).
(The V bias contributes b_v @ w_o to the output because softmax rows sum
to 1; the out-proj bias is b_o. Both are token-independent row vectors.)

The host pre-transposes x (xT [1024, 2048]) and casts x + all weights to
bf16, so the device program has no transpose phase and all matmuls run
bf16 at 1 cycle/row (measured end-to-end precision ~5e-3 vs the 2e-2
gate). All weights are SBUF-resident up front (bf16 halves the footprint)
so no weight streaming happens during attention.

On-core layouts:
  xt   [1024,2048] bf16  feature-major x (8 tiles [128,2048])
  qt/kt[512 ,2048] bf16  feature-major; tile g holds heads 2g,2g+1
  v_aug[2048, 520] bf16  token-major, 65 cols/head: 64 V cols + a ones
                         column (makes the PV matmul also produce the
                         softmax denominator as PSUM row 64)
  scores ST [nk,mq] per 128-row tile; exp on ACT (scale=1/8, no max
                         subtraction -- scores are ~N(0,1))
  causal mask            affine_select (GPSIMD) zeroes exp() only on the
                         128 diagonal columns of diagonal tiles; fully
                         masked column prefixes are skipped entirely
  normalization          PSUM evacuated to SBUF bf16 immediately (frees
                         the accumulation bank), then reciprocal of the
                         sum row + PE ones-broadcast + DVE mul
The attention inner loop is software-pipelined: score(t+1) is emitted
before PV(t) so the PE never head-of-line blocks on the exp of tile t.
"""

import sys

if "/opt/trn_rl_repo" not in sys.path:
    sys.path.insert(0, "/opt/trn_rl_repo")

import numpy as np

import concourse.bass as bass
import concourse.tile as tile
from concourse import bacc, mybir
from concourse.bass_utils import run_bass_kernel_spmd

N_CORES = 8
S = 2048
D = 1024
DH = 512          # per-core feature width (8 heads x 64)
HD = 64           # head dim
NH_LOC = 8        # heads per core
F32 = mybir.dt.float32
BF16 = mybir.dt.bfloat16
EXP = mybir.ActivationFunctionType.Exp
GE = mybir.AluOpType.is_ge

_PROGRAM = None


def _build_program(n_repeat=1):
    nc = bacc.Bacc("TRN2", target_bir_lowering=False, debug=False,
                   num_devices=N_CORES)
    xt_d = nc.dram_tensor("xt", [D, S], BF16, kind="ExternalInput").ap()
    wq_d = nc.dram_tensor("wq", [D, DH], BF16, kind="ExternalInput").ap()
    wk_d = nc.dram_tensor("wk", [D, DH], BF16, kind="ExternalInput").ap()
    wv_d = nc.dram_tensor("wv", [D, DH], BF16, kind="ExternalInput").ap()
    wo_d = nc.dram_tensor("wo", [DH, D], BF16, kind="ExternalInput").ap()
    bq_d = nc.dram_tensor("bq", [DH], F32, kind="ExternalInput").ap()
    bk_d = nc.dram_tensor("bk", [DH], F32, kind="ExternalInput").ap()
    part_d = nc.dram_tensor("part", [S, D], BF16, kind="ExternalOutput").ap()

    with tile.TileContext(nc) as tc:
        for _ in range(n_repeat):
            _emit(nc, tc, xt_d, wq_d, wk_d, wv_d, wo_d, bq_d, bk_d, part_d)
    nc.compile()
    return nc


def _emit(nc, tc, xt_d, wq_d, wk_d, wv_d, wo_d, bq_d, bk_d, part_d):
    """Emission is hand-pipelined: per-engine instruction order follows
    emission order. Skeleton:

      [DMA biases, xt/wq/wk interleaved, wv, wo]
      [Q0/K0 projection]                       (starts as DMAs land)
      [V projection mt 0..3]
      [attention pair0 || Vproj mt 4..15 + Q1/K1]
      [attention pair1 || Q2/K2 projection]
      [attention pair2 || Q3/K3 projection]
      [attention pair3 || per-chunk out-projection]
    """
    from contextlib import ExitStack

    es = ExitStack()
    with es:
        const_pool = es.enter_context(tc.tile_pool(name="const", bufs=1))
        w_pool = es.enter_context(tc.tile_pool(name="wp", bufs=1))
        v_pool = es.enter_context(tc.tile_pool(name="vaug", bufs=1))
        qkv_pool = es.enter_context(tc.tile_pool(name="qkv", bufs=2))
        exp_pool = es.enter_context(tc.tile_pool(name="ex", bufs=6))
        aun_pool = es.enter_context(tc.tile_pool(name="aun", bufs=4))
        rec_pool = es.enter_context(tc.tile_pool(name="rc", bufs=4))
        ot_pool = es.enter_context(tc.tile_pool(name="otl", bufs=16))
        os_pool = es.enter_context(tc.tile_pool(name="os", bufs=4))
        st_pool = es.enter_context(
            tc.tile_pool(name="st", bufs=2, space="PSUM"))
        pp_pool = es.enter_context(
            tc.tile_pool(name="pp", bufs=2, space="PSUM"))
        otp_pool = es.enter_context(
            tc.tile_pool(name="ops", bufs=2, space="PSUM"))

        ones_bf = const_pool.tile([128, HD], BF16, name="ones_bf",
                                  tag="ones")
        nc.vector.memset(ones_bf, 1.0)
        bq_sb = const_pool.tile([128, 4], F32, name="bq_sb", tag="bq")
        bk_sb = const_pool.tile([128, 4], F32, name="bk_sb", tag="bk")

        # ---- bulk DMAs (biases first: tiny, needed by Q0/K0) ----------
        for g in range(4):
            sl = slice(g * 128, (g + 1) * 128)
            nc.sync.dma_start(
                out=bq_sb[:, g:g + 1],
                in_=bq_d[sl].rearrange("(p one) -> p one", one=1))
            nc.sync.dma_start(
                out=bk_sb[:, g:g + 1],
                in_=bk_d[sl].rearrange("(p one) -> p one", one=1))
        xt = []
        wq_sb = []
        wk_sb = []
        wv_sb = []
        for kc in range(8):
            rows = slice(kc * 128, (kc + 1) * 128)
            x_t = w_pool.tile([128, S], BF16, name=f"xt{kc}", tag=f"xt{kc}")
            nc.sync.dma_start(out=x_t, in_=xt_d[rows, :])
            xt.append(x_t)
            wq_t = w_pool.tile([128, DH], BF16, name=f"wq{kc}",
                               tag=f"wq{kc}")
            nc.sync.dma_start(out=wq_t, in_=wq_d[rows, :])
            wq_sb.append(wq_t)
            wk_t = w_pool.tile([128, DH], BF16, name=f"wk{kc}",
                               tag=f"wk{kc}")
            nc.sync.dma_start(out=wk_t, in_=wk_d[rows, :])
            wk_sb.append(wk_t)
        for kc in range(8):
            rows = slice(kc * 128, (kc + 1) * 128)
            wv_t = w_pool.tile([128, DH], BF16, name=f"wv{kc}",
                               tag=f"wv{kc}")
            nc.sync.dma_start(out=wv_t, in_=wv_d[rows, :])
            wv_sb.append(wv_t)
        wo_sb = []
        for fc in range(4):
            wo_t = w_pool.tile([128, D], BF16, name=f"wo{fc}", tag=f"wo{fc}")
            nc.sync.dma_start(
                out=wo_t, in_=wo_d[fc * 128:(fc + 1) * 128, :])
            wo_sb.append(wo_t)

        v_aug = [v_pool.tile([128, NH_LOC * (HD + 1)], BF16,
                             name=f"va{t}", tag=f"va{t}")
                 for t in range(16)]

        qt = {}
        kt = {}
        ot_map = {}

        def emit_vproj_unit(mt):
            pp = pp_pool.tile([128, 512], F32, name=f"ppv{mt}", tag="pp")
            for kc in range(8):
                nc.tensor.matmul(
                    pp[:],
                    xt[kc][:, mt * 128:(mt + 1) * 128],
                    wv_sb[kc][:],
                    start=(kc == 0), stop=(kc == 7))
            va3 = v_aug[mt].rearrange("p (h c) -> p h c", h=NH_LOC)
            nc.vector.tensor_copy(
                va3[:, :, 0:HD],
                pp[:].rearrange("p (h c) -> p h c", h=NH_LOC))
            nc.vector.tensor_copy(
                va3[:, :, HD:HD + 1],
                ones_bf[:, 0:NH_LOC].rearrange(
                    "p (h one) -> p h one", one=1))

        def emit_proj_units(g):
            """Q/K projection for pair g as a list of emission closures."""
            units = []

            def u_alloc_q():
                qt[g] = qkv_pool.tile([128, S], BF16, name=f"qt{g}",
                                      tag="qt")

            def u_alloc_k():
                kt[g] = qkv_pool.tile([128, S], BF16, name=f"kt{g}",
                                      tag="kt")

            units.append(u_alloc_q)
            units.append(u_alloc_k)

            def mk_mc(which, w_sb, bias_sb, mc):
                def u():
                    out_t = qt[g] if which == "qt" else kt[g]
                    pp = pp_pool.tile([128, 512], F32,
                                      name=f"pp{which}{g}_{mc}", tag="pp")
                    for kc in range(8):
                        nc.tensor.matmul(
                            pp[:],
                            w_sb[kc][:, g * 128:(g + 1) * 128],
                            xt[kc][:, mc * 512:(mc + 1) * 512],
                            start=(kc == 0), stop=(kc == 7))
                    nc.vector.tensor_scalar_add(
                        out_t[:, mc * 512:(mc + 1) * 512],
                        pp[:], bias_sb[:, g:g + 1])
                return u

            for mc in range(4):
                units.append(mk_mc("qt", wq_sb, bq_sb, mc))
            for mc in range(4):
                units.append(mk_mc("kt", wk_sb, bk_sb, mc))
            return units

        def emit_score(g, j, t):
            # Columns [0, z) of a diagonal tile are fully masked: skip them
            # in the score matmul, exp, mask, and PV accumulation entirely
            # (the PV start=True tile always covers the full width, so the
            # untouched PSUM columns keep their accumulated values).
            d = t - 4 * j
            z = 0 if d < 0 else 128 * d
            w = 512 - z
            mq = slice(j * 512 + z, (j + 1) * 512)
            nk = slice(t * 128, (t + 1) * 128)
            st = st_pool.tile([128, 1024], F32,
                              name=f"st{j}_{g}_{t}", tag="st")
            for hl in range(2):
                dsl = slice(hl * 64, hl * 64 + 64)
                nc.tensor.matmul(
                    st[:, hl * 512:hl * 512 + w],
                    kt[g][dsl, nk], qt[g][dsl, mq],
                    start=True, stop=True)
            ex = exp_pool.tile([128, 1024], BF16,
                               name=f"ex{j}_{g}_{t}", tag="ex")
            if d < 0:
                # off-diagonal: both head halves in one wide ACT op
                nc.scalar.activation(ex[:, 0:1024], st[:, 0:1024],
                                     EXP, scale=0.125)
            else:
                # diagonal: strided 3D exp over both heads, then the causal
                # mask only needs the first 128 columns (the rest of the
                # tile is strictly below the diagonal and fully kept)
                st3 = st.rearrange("p (h q) -> p h q", h=2)[:, :, 0:w]
                ex3 = ex.rearrange("p (h q) -> p h q", h=2)[:, :, 0:w]
                nc.scalar.activation(ex3, st3, EXP, scale=0.125)
                # keep where local_mq >= local_nk: y - p >= 0
                nc.gpsimd.affine_select(
                    out=ex3[:, :, 0:128], in_=ex3[:, :, 0:128],
                    compare_op=GE, fill=0.0, base=0,
                    channel_multiplier=-1,
                    pattern=[[0, 2], [1, 128]])
            return ex, z, w

        def emit_pv(g, j, t, t_max, ex, z, w):
            o_ps = ot_map[("ps", g, j)]
            for hl in range(2):
                h = 2 * g + hl
                nc.tensor.matmul(
                    o_ps[hl][:, z:512],
                    v_aug[t][:, 65 * h:65 * h + 65],
                    ex[:, hl * 512:hl * 512 + w],
                    start=(t == 0), stop=(t == t_max - 1))

        def emit_normalize(g, j):
            o_ps = ot_map.pop(("ps", g, j))
            ot_t = ot_pool.tile([128, 512], BF16,
                                name=f"ot{j}_{g}", tag="ot")
            ot_map[(j, g)] = ot_t
            for hl in range(2):
                # Evacuate PSUM to SBUF bf16 right away to free the
                # accumulation bank, then 1/sum on the sum row, broadcast
                # across partitions with a rank-1 PE matmul, and scale.
                aun = aun_pool.tile([65, 512], BF16,
                                    name=f"aun{j}_{g}_{hl}", tag="aun")
                nc.vector.tensor_copy(aun[:], o_ps[hl][:])
                rec = rec_pool.tile([1, 512], BF16,
                                    name=f"rc{j}_{g}_{hl}", tag="rc")
                with nc.allow_low_precision(reason="bf16 recip row"):
                    nc.vector.reciprocal(rec[:], aun[64:65, :])
                bc = pp_pool.tile([64, 512], F32,
                                  name=f"bc{j}_{g}_{hl}", tag="pp")
                nc.tensor.matmul(bc[:], ones_bf[0:1, 0:HD],
                                 rec[0:1, :], start=True, stop=True)
                nc.vector.tensor_mul(
                    ot_t[64 * hl:64 * hl + 64, :],
                    aun[0:64, :], bc[:])

        def att_iters_for_pair(g):
            """Software-pipelined: unit(t) = score(t)+exp(t), PV(t-1)."""
            iters = []
            for j in range(4):
                t_max = 4 * (j + 1)
                state = {}

                def mk_first(g=g, j=j, state=state):
                    def u():
                        ot_map[("ps", g, j)] = [
                            otp_pool.tile([65, 512], F32,
                                          name=f"o{j}_{g}_{hl}", tag="ops")
                            for hl in range(2)]
                        state[0] = emit_score(g, j, 0)
                    return u
                iters.append(mk_first())
                for t in range(1, t_max):
                    def mk(g=g, j=j, t=t, t_max=t_max, state=state):
                        def u():
                            state[t] = emit_score(g, j, t)
                            ex, z, w = state.pop(t - 1)
                            emit_pv(g, j, t - 1, t_max, ex, z, w)
                        return u
                    iters.append(mk())

                def mk_last(g=g, j=j, t_max=t_max, state=state):
                    def u():
                        ex, z, w = state.pop(t_max - 1)
                        emit_pv(g, j, t_max - 1, t_max, ex, z, w)
                    return u
                iters.append(mk_last())

                def mk_norm(g=g, j=j):
                    def u():
                        emit_normalize(g, j)
                    return u
                iters.append(mk_norm())
            return iters

        def zip_emit(primary, filler):
            n_p, n_f = len(primary), len(filler)
            fi = 0
            for i, p in enumerate(primary):
                p()
                while fi < n_f and (i + 1) * n_f >= (fi + 1) * n_p:
                    filler[fi]()
                    fi += 1
            while fi < n_f:
                filler[fi]()
                fi += 1

        def emit_outproj_one(j, mt, nck):
            msl = slice((mt - 4 * j) * 128, (mt - 4 * j) * 128 + 128)
            op = pp_pool.tile([128, 512], F32,
                              name=f"op{mt}_{nck}", tag="pp")
            for g in range(4):
                nc.tensor.matmul(
                    op[:],
                    ot_map[(j, g)][:, msl],
                    wo_sb[g][:, nck * 512:(nck + 1) * 512],
                    start=(g == 0), stop=(g == 3))
            osb = os_pool.tile([128, 512], BF16,
                               name=f"os{mt}_{nck}", tag="os")
            nc.vector.tensor_copy(osb[:], op[:])
            nc.sync.dma_start(
                out=part_d[mt * 128:(mt + 1) * 128,
                           nck * 512:(nck + 1) * 512],
                in_=osb[:])

        def outproj_units(j):
            units = []
            for mt in range(4 * j, 4 * j + 4):
                for nck in range(2):
                    def u(j=j, mt=mt, nck=nck):
                        emit_outproj_one(j, mt, nck)
                    units.append(u)
            return units

        # Q0/K0 first (paces on the xt/wq/wk DMA stream), then the first
        # four V tiles (enough for attention chunk j=0), then attention(g)
        # zipped with the trailing V tiles and later projections.
        for u in emit_proj_units(0):
            u()
        for mt in range(4):
            emit_vproj_unit(mt)

        def v_tail_units():
            units = []
            for mt in range(4, 16):
                def u(mt=mt):
                    emit_vproj_unit(mt)
                return_u = u
                units.append(return_u)
            return units

        zip_emit(att_iters_for_pair(0), v_tail_units() + emit_proj_units(1))
        for g in range(1, 3):
            zip_emit(att_iters_for_pair(g), emit_proj_units(g + 1))

        # Last pair: chunk j-1's out-projection is zipped between chunk j's
        # attention iterations so the PE keeps filler work while the
        # normalize chain drains; chunk 3's out-projection closes the tail.
        g = 3
        att3 = att_iters_for_pair(g)
        # per-j unit counts: t_max + 2
        ofs = 0
        for j in range(4):
            cnt = 4 * (j + 1) + 2
            zip_emit(att3[ofs:ofs + cnt],
                     outproj_units(j - 1) if j > 0 else [])
            ofs += cnt
        for u in outproj_units(3):
            u()


def _get_program():
    global _PROGRAM
    if _PROGRAM is None:
        _PROGRAM = _build_program()
    return _PROGRAM


_EXEC = None


def _get_executor():
    """Build the sharded PJRT executable once and reuse it across calls.

    Mirrors bass2jax.run_bass_via_pjrt's multi-core branch, but caches the
    jitted callable so repeat kernel() calls skip retracing/recompilation.
    Returns (fn, fn_nodonate, in_names, out_names, out_shapes, out_dtypes,
    mesh). fn takes globally concatenated inputs (n_cores*dim0, ...) plus
    donated zero output buffers, and returns concatenated outputs.
    """
    global _EXEC
    if _EXEC is None:
        import jax
        from jax.experimental.shard_map import shard_map
        from jax.sharding import Mesh, PartitionSpec

        from concourse import bass2jax

        bass2jax.install_neuronx_cc_hook()
        nc = _get_program()
        part_name = (nc.partition_id_tensor.name
                     if nc.partition_id_tensor else None)
        in_names, out_names, out_avals = [], [], []
        for alloc in nc.m.functions[0].allocations:
            if not isinstance(alloc, mybir.MemoryLocationSet):
                continue
            name = alloc.memorylocations[0].name
            if alloc.kind == "ExternalInput":
                if name != part_name:
                    in_names.append(name)
            elif alloc.kind == "ExternalOutput":
                out_names.append(name)
                out_avals.append(jax.core.ShapedArray(
                    tuple(alloc.tensor_shape), mybir.dt.np(alloc.dtype)))
        n_params = len(in_names)
        all_in = tuple(in_names) + tuple(out_names)
        if part_name is not None:
            all_in = all_in + (part_name,)

        def _body(*args):
            operands = list(args)
            if part_name is not None:
                operands.append(bass2jax.partition_id_tensor())
            outs = bass2jax._bass_exec_p.bind(
                *operands,
                out_avals=tuple(out_avals),
                in_names=all_in,
                out_names=tuple(out_names),
                lowering_input_output_aliases=(),
                sim_require_finite=True,
                sim_require_nnan=True,
                nc=nc)
            return tuple(outs)

        devices = jax.devices()[:N_CORES]
        mesh = Mesh(np.asarray(devices), ("core",))
        n_bufs = n_params + len(out_names)
        mapped = shard_map(_body, mesh=mesh,
                           in_specs=(PartitionSpec("core"),) * n_bufs,
                           out_specs=(PartitionSpec("core"),) * len(out_names),
                           check_rep=False)
        fn = jax.jit(mapped,
                     donate_argnums=tuple(range(n_params, n_bufs)),
                     keep_unused=True)
        # Non-donating twin: lets a timing loop reuse device-resident
        # argument buffers across calls (we write every element of every
        # output, so uninitialized result buffers are fine).
        fn_nodonate = jax.jit(mapped, keep_unused=True)
        out_shapes = [tuple(a.shape) for a in out_avals]
        out_dtypes = [a.dtype for a in out_avals]
        _EXEC = (fn, fn_nodonate, in_names, out_names, out_shapes,
                 out_dtypes, mesh)
    return _EXEC


def run_cores(in_maps):
    """Run the SPMD program on 8 cores via the cached executable."""
    fn, _, in_names, out_names, out_shapes, out_dtypes = _get_executor()[:6]
    concat_in = [np.concatenate([in_maps[c][n] for c in range(N_CORES)],
                                axis=0) for n in in_names]
    zeros = [np.zeros((N_CORES * s[0],) + s[1:], dt)
             for s, dt in zip(out_shapes, out_dtypes)]
    outs = fn(*concat_in, *zeros)
    res = []
    for c in range(N_CORES):
        res.append({
            n: np.asarray(outs[i]).reshape((N_CORES,) + out_shapes[i])[c]
            for i, n in enumerate(out_names)})
    return res


def make_in_maps(x, w_q, b_q, w_k, b_k, w_v, b_v, w_o, b_o):
    import ml_dtypes

    bf = ml_dtypes.bfloat16
    in_maps = []
    for c in range(N_CORES):
        b, hh = divmod(c, 2)
        cols = slice(hh * DH, (hh + 1) * DH)
        in_maps.append({
            "xt": np.ascontiguousarray(x[b].T).astype(bf),
            "wq": np.ascontiguousarray(w_q[:, cols]).astype(bf),
            "wk": np.ascontiguousarray(w_k[:, cols]).astype(bf),
            "wv": np.ascontiguousarray(w_v[:, cols]).astype(bf),
            "wo": np.ascontiguousarray(w_o[cols, :]).astype(bf),
            "bq": np.ascontiguousarray(b_q[cols]),
            "bk": np.ascontiguousarray(b_k[cols]),
        })
    return in_maps


def combine(parts, b_v, w_o, b_o):
    corr = (b_v @ w_o + b_o).astype(np.float32)
    out = np.empty((4, S, D), dtype=np.float32)
    for b in range(4):
        out[b] = (parts[2 * b].astype(np.float32)
                  + parts[2 * b + 1].astype(np.float32) + corr)
    return out


def kernel(x, w_q, b_q, w_k, b_k, w_v, b_v, w_o, b_o):
    x = np.asarray(x, dtype=np.float32)
    w_q = np.asarray(w_q, dtype=np.float32)
    b_q = np.asarray(b_q, dtype=np.float32)
    w_k = np.asarray(w_k, dtype=np.float32)
    b_k = np.asarray(b_k, dtype=np.float32)
    w_v = np.asarray(w_v, dtype=np.float32)
    b_v = np.asarray(b_v, dtype=np.float32)
    w_o = np.asarray(w_o, dtype=np.float32)
    b_o = np.asarray(b_o, dtype=np.float32)

    in_maps = make_in_maps(x, w_q, b_q, w_k, b_k, w_v, b_v, w_o, b_o)
    res = run_cores(in_maps)
    parts = [res[c]["part"] for c in range(N_CORES)]
    return combine(parts, b_v, w_o, b_o)


# revision 9
# speedup vs baseline: 1.5141x; 1.3650x over previous
"""Causal self-attention (B=4, S=2048, D=1024, H=16) on 8 trn2 cores.

Sharding: core c -> (batch b = c//2, head-half hh = c%2). Each core:
  - computes Q/K/V projections for its batch restricted to its 8 heads
    (512 of the 1024 feature columns),
  - runs causal attention for those heads,
  - computes a partial out-projection part = attnO @ w_o[rows of its heads].
Host: out[b] = part[2b] + part[2b+1] + (b_v @ w_o + b_o).
(The V bias contributes b_v @ w_o to the output because softmax rows sum
to 1; the out-proj bias is b_o. Both are token-independent row vectors.)

The host pre-transposes x (xT [1024, 2048]) and casts x + all weights to
bf16, so the device program has no transpose phase and all matmuls run
bf16 at 1 cycle/row (measured end-to-end precision ~5e-3 vs the 2e-2
gate). All weights are SBUF-resident up front (bf16 halves the footprint)
so no weight streaming happens during attention.

On-core layouts:
  xt   [1024,2048] bf16  feature-major x (8 tiles [128,2048])
  qt/kt[512 ,2048] bf16  feature-major; tile g holds heads 2g,2g+1
  v_aug[2048, 520] bf16  token-major, 65 cols/head: 64 V cols + a ones
                         column (makes the PV matmul also produce the
                         softmax denominator as PSUM row 64)
  scores ST [nk,mq] per 128-row tile; exp on ACT (scale=1/8, no max
                         subtraction -- scores are ~N(0,1))
  causal mask            affine_select (GPSIMD) zeroes exp() only on the
                         128 diagonal columns of diagonal tiles; fully
                         masked column prefixes are skipped entirely
  normalization          PSUM evacuated to SBUF bf16 immediately (frees
                         the accumulation bank), then reciprocal of the
                         sum row + PE ones-broadcast + DVE mul
The attention inner loop is software-pipelined: score(t+1) is emitted
before PV(t) so the PE never head-of-line blocks on the exp of tile t.
"""

import sys

if "/opt/trn_rl_repo" not in sys.path:
    sys.path.insert(0, "/opt/trn_rl_repo")

import numpy as np

import concourse.bass as bass
import concourse.tile as tile
from concourse import bacc, mybir
from concourse.bass_utils import run_bass_kernel_spmd

N_CORES = 8
S = 2048
D = 1024
DH = 512          # per-core feature width (8 heads x 64)
HD = 64           # head dim
NH_LOC = 8        # heads per core
F32 = mybir.dt.float32
BF16 = mybir.dt.bfloat16
EXP = mybir.ActivationFunctionType.Exp
GE = mybir.AluOpType.is_ge

_PROGRAM = None


def _build_program(n_repeat=1):
    nc = bacc.Bacc("TRN2", target_bir_lowering=False, debug=False,
                   num_devices=N_CORES)
    xt_d = nc.dram_tensor("xt", [D, S], BF16, kind="ExternalInput").ap()
    wq_d = nc.dram_tensor("wq", [D, DH], BF16, kind="ExternalInput").ap()
    wk_d = nc.dram_tensor("wk", [D, DH], BF16, kind="ExternalInput").ap()
    wv_d = nc.dram_tensor("wv", [D, DH], BF16, kind="ExternalInput").ap()
    wo_d = nc.dram_tensor("wo", [DH, D], BF16, kind="ExternalInput").ap()
    bq_d = nc.dram_tensor("bq", [DH], F32, kind="ExternalInput").ap()
    bk_d = nc.dram_tensor("bk", [DH], F32, kind="ExternalInput").ap()
    part_d = nc.dram_tensor("part", [S, D], BF16, kind="ExternalOutput").ap()

    with tile.TileContext(nc) as tc:
        for _ in range(n_repeat):
            _emit(nc, tc, xt_d, wq_d, wk_d, wv_d, wo_d, bq_d, bk_d, part_d)
    nc.compile()
    return nc


def _emit(nc, tc, xt_d, wq_d, wk_d, wv_d, wo_d, bq_d, bk_d, part_d):
    """Emission is hand-pipelined: per-engine instruction order follows
    emission order. Skeleton:

      [DMA biases, xt/wq/wk interleaved, wv, wo]
      [Q0/K0 projection]                       (starts as DMAs land)
      [V projection mt 0..3]
      [attention pair0 || Vproj mt 4..15 + Q1/K1]
      [attention pair1 || Q2/K2 projection]
      [attention pair2 || Q3/K3 projection]
      [attention pair3 || per-chunk out-projection]
    """
    from contextlib import ExitStack

    es = ExitStack()
    with es:
        const_pool = es.enter_context(tc.tile_pool(name="const", bufs=1))
        w_pool = es.enter_context(tc.tile_pool(name="wp", bufs=1))
        v_pool = es.enter_context(tc.tile_pool(name="vaug", bufs=1))
        qkv_pool = es.enter_context(tc.tile_pool(name="qkv", bufs=2))
        exp_pool = es.enter_context(tc.tile_pool(name="ex", bufs=6))
        aun_pool = es.enter_context(tc.tile_pool(name="aun", bufs=4))
        rec_pool = es.enter_context(tc.tile_pool(name="rc", bufs=4))
        ot_pool = es.enter_context(tc.tile_pool(name="otl", bufs=16))
        os_pool = es.enter_context(tc.tile_pool(name="os", bufs=4))
        st_pool = es.enter_context(
            tc.tile_pool(name="st", bufs=2, space="PSUM"))
        pp_pool = es.enter_context(
            tc.tile_pool(name="pp", bufs=2, space="PSUM"))
        otp_pool = es.enter_context(
            tc.tile_pool(name="ops", bufs=2, space="PSUM"))

        ones_bf = const_pool.tile([128, HD], BF16, name="ones_bf",
                                  tag="ones")
        nc.vector.memset(ones_bf, 1.0)
        bq_sb = const_pool.tile([128, 4], F32, name="bq_sb", tag="bq")
        bk_sb = const_pool.tile([128, 4], F32, name="bk_sb", tag="bk")

        # ---- bulk DMAs (biases first: tiny, needed by Q0/K0) ----------
        # Inputs go through the ACT engine's HWDGE queue so that, across
        # repeats, the next body's input stream is not serialized behind
        # this body's part writebacks on the SP queue (ACT's own exp work
        # is done well before the body tail, so the queue is free).
        # The SP queue still holds the previous body's part writebacks at
        # this point, so it gets only the late-needed inputs (biases first:
        # tiny, then wv/wo). The big early-needed stream (xt/wq/wk) rides
        # the ACT queue, which frees right after the previous body's last
        # exp -- ~25us before the body boundary -- giving real prefetch.
        for g in range(4):
            sl = slice(g * 128, (g + 1) * 128)
            nc.scalar.dma_start(
                out=bq_sb[:, g:g + 1],
                in_=bq_d[sl].rearrange("(p one) -> p one", one=1))
            nc.scalar.dma_start(
                out=bk_sb[:, g:g + 1],
                in_=bk_d[sl].rearrange("(p one) -> p one", one=1))
        xt = []
        wq_sb = []
        wk_sb = []
        wv_sb = []
        for kc in range(8):
            rows = slice(kc * 128, (kc + 1) * 128)
            x_t = w_pool.tile([128, S], BF16, name=f"xt{kc}", tag=f"xt{kc}")
            nc.scalar.dma_start(out=x_t, in_=xt_d[rows, :])
            xt.append(x_t)
            wq_t = w_pool.tile([128, DH], BF16, name=f"wq{kc}",
                               tag=f"wq{kc}")
            nc.scalar.dma_start(out=wq_t, in_=wq_d[rows, :])
            wq_sb.append(wq_t)
            wk_t = w_pool.tile([128, DH], BF16, name=f"wk{kc}",
                               tag=f"wk{kc}")
            nc.scalar.dma_start(out=wk_t, in_=wk_d[rows, :])
            wk_sb.append(wk_t)
        for kc in range(8):
            rows = slice(kc * 128, (kc + 1) * 128)
            wv_t = w_pool.tile([128, DH], BF16, name=f"wv{kc}",
                               tag=f"wv{kc}")
            nc.scalar.dma_start(out=wv_t, in_=wv_d[rows, :])
            wv_sb.append(wv_t)
        wo_sb = []
        for fc in range(4):
            wo_t = w_pool.tile([128, D], BF16, name=f"wo{fc}", tag=f"wo{fc}")
            nc.sync.dma_start(
                out=wo_t, in_=wo_d[fc * 128:(fc + 1) * 128, :])
            wo_sb.append(wo_t)

        v_aug = [v_pool.tile([128, NH_LOC * (HD + 1)], BF16,
                             name=f"va{t}", tag=f"va{t}")
                 for t in range(16)]

        qt = {}
        kt = {}
        ot_map = {}

        def emit_vproj_unit(mt):
            pp = pp_pool.tile([128, 512], F32, name=f"ppv{mt}", tag="pp")
            for kc in range(8):
                nc.tensor.matmul(
                    pp[:],
                    xt[kc][:, mt * 128:(mt + 1) * 128],
                    wv_sb[kc][:],
                    start=(kc == 0), stop=(kc == 7))
            va3 = v_aug[mt].rearrange("p (h c) -> p h c", h=NH_LOC)
            nc.vector.tensor_copy(
                va3[:, :, 0:HD],
                pp[:].rearrange("p (h c) -> p h c", h=NH_LOC))
            nc.vector.tensor_copy(
                va3[:, :, HD:HD + 1],
                ones_bf[:, 0:NH_LOC].rearrange(
                    "p (h one) -> p h one", one=1))

        def emit_proj_units(g):
            """Q/K projection for pair g as a list of emission closures."""
            units = []

            def u_alloc_q():
                qt[g] = qkv_pool.tile([128, S], BF16, name=f"qt{g}",
                                      tag="qt")

            def u_alloc_k():
                kt[g] = qkv_pool.tile([128, S], BF16, name=f"kt{g}",
                                      tag="kt")

            units.append(u_alloc_q)
            units.append(u_alloc_k)

            def mk_mc(which, w_sb, bias_sb, mc):
                def u():
                    out_t = qt[g] if which == "qt" else kt[g]
                    pp = pp_pool.tile([128, 512], F32,
                                      name=f"pp{which}{g}_{mc}", tag="pp")
                    for kc in range(8):
                        nc.tensor.matmul(
                            pp[:],
                            w_sb[kc][:, g * 128:(g + 1) * 128],
                            xt[kc][:, mc * 512:(mc + 1) * 512],
                            start=(kc == 0), stop=(kc == 7))
                    nc.vector.tensor_scalar_add(
                        out_t[:, mc * 512:(mc + 1) * 512],
                        pp[:], bias_sb[:, g:g + 1])
                return u

            for mc in range(4):
                units.append(mk_mc("qt", wq_sb, bq_sb, mc))
            for mc in range(4):
                units.append(mk_mc("kt", wk_sb, bk_sb, mc))
            return units

        def emit_score(g, j, t):
            # Columns [0, z) of a diagonal tile are fully masked: skip them
            # in the score matmul, exp, mask, and PV accumulation entirely
            # (the PV start=True tile always covers the full width, so the
            # untouched PSUM columns keep their accumulated values).
            d = t - 4 * j
            z = 0 if d < 0 else 128 * d
            w = 512 - z
            mq = slice(j * 512 + z, (j + 1) * 512)
            nk = slice(t * 128, (t + 1) * 128)
            st = st_pool.tile([128, 1024], F32,
                              name=f"st{j}_{g}_{t}", tag="st")
            for hl in range(2):
                dsl = slice(hl * 64, hl * 64 + 64)
                nc.tensor.matmul(
                    st[:, hl * 512:hl * 512 + w],
                    kt[g][dsl, nk], qt[g][dsl, mq],
                    start=True, stop=True)
            ex = exp_pool.tile([128, 1024], BF16,
                               name=f"ex{j}_{g}_{t}", tag="ex")
            if d < 0:
                # off-diagonal: both head halves in one wide ACT op
                nc.scalar.activation(ex[:, 0:1024], st[:, 0:1024],
                                     EXP, scale=0.125)
            else:
                # diagonal: strided 3D exp over both heads, then the causal
                # mask only needs the first 128 columns (the rest of the
                # tile is strictly below the diagonal and fully kept)
                st3 = st.rearrange("p (h q) -> p h q", h=2)[:, :, 0:w]
                ex3 = ex.rearrange("p (h q) -> p h q", h=2)[:, :, 0:w]
                nc.scalar.activation(ex3, st3, EXP, scale=0.125)
                # keep where local_mq >= local_nk: y - p >= 0
                nc.gpsimd.affine_select(
                    out=ex3[:, :, 0:128], in_=ex3[:, :, 0:128],
                    compare_op=GE, fill=0.0, base=0,
                    channel_multiplier=-1,
                    pattern=[[0, 2], [1, 128]])
            return ex, z, w

        def emit_pv(g, j, t, t_max, ex, z, w):
            o_ps = ot_map[("ps", g, j)]
            for hl in range(2):
                h = 2 * g + hl
                nc.tensor.matmul(
                    o_ps[hl][:, z:512],
                    v_aug[t][:, 65 * h:65 * h + 65],
                    ex[:, hl * 512:hl * 512 + w],
                    start=(t == 0), stop=(t == t_max - 1))

        def emit_normalize(g, j):
            o_ps = ot_map.pop(("ps", g, j))
            ot_t = ot_pool.tile([128, 512], BF16,
                                name=f"ot{j}_{g}", tag="ot")
            ot_map[(j, g)] = ot_t
            for hl in range(2):
                # Evacuate PSUM to SBUF bf16 right away (on GPSIMD, which
                # is mostly idle) to free the accumulation bank, then 1/sum
                # on the sum row, broadcast across partitions with GPSIMD
                # partition_broadcast, and scale. The chain never touches
                # the PE, so the next chunk's scores are not head-of-line
                # blocked behind it.
                aun = aun_pool.tile([65, 512], BF16,
                                    name=f"aun{j}_{g}_{hl}", tag="aun")
                nc.vector.tensor_copy(aun[:], o_ps[hl][:])
                rec = rec_pool.tile([1, 512], BF16,
                                    name=f"rc{j}_{g}_{hl}", tag="rc")
                with nc.allow_low_precision(reason="bf16 recip row"):
                    nc.vector.reciprocal(rec[:], aun[64:65, :])
                bc = aun_pool.tile([64, 512], BF16,
                                   name=f"bc{j}_{g}_{hl}", tag="bc")
                nc.gpsimd.partition_broadcast(bc[:], rec[0:1, :])
                nc.vector.tensor_mul(
                    ot_t[64 * hl:64 * hl + 64, :],
                    aun[0:64, :], bc[:])

        def att_iters_for_pair(g):
            """Software-pipelined continuously across chunk boundaries:
            unit(s) = score(s)+exp(s), then PV(s-1) (+normalize when s-1
            closed its chunk). The PE never queues directly behind the
            exp/mask of the tile it just scored."""
            seq = [(j, t) for j in range(4) for t in range(4 * (j + 1))]
            state = {}
            iters = []

            def mk(idx):
                j, t = seq[idx]

                def u():
                    if t == 0:
                        ot_map[("ps", g, j)] = [
                            otp_pool.tile([65, 512], F32,
                                          name=f"o{j}_{g}_{hl}", tag="ops")
                            for hl in range(2)]
                    state[idx] = emit_score(g, j, t)
                    if idx > 0:
                        pj, pt = seq[idx - 1]
                        ex, z, w = state.pop(idx - 1)
                        emit_pv(g, pj, pt, 4 * (pj + 1), ex, z, w)
                        if pt == 4 * (pj + 1) - 1:
                            emit_normalize(g, pj)
                return u

            for idx in range(len(seq)):
                iters.append(mk(idx))

            def u_last():
                ex, z, w = state.pop(len(seq) - 1)
                emit_pv(g, 3, 15, 16, ex, z, w)
                emit_normalize(g, 3)
            iters.append(u_last)
            return iters

        def zip_emit(primary, filler, frac=0.7):
            # Spread fillers over the first `frac` of the primary units so
            # their results (e.g. qt/kt of the next pair) are ready before
            # the primary stream needs them.
            n_p, n_f = len(primary), len(filler)
            n_eff = max(1, int(n_p * frac))
            fi = 0
            for i, p in enumerate(primary):
                p()
                while fi < n_f and (i + 1) * n_f >= (fi + 1) * n_eff:
                    filler[fi]()
                    fi += 1
            while fi < n_f:
                filler[fi]()
                fi += 1

        def emit_outproj_one(j, mt, nck):
            msl = slice((mt - 4 * j) * 128, (mt - 4 * j) * 128 + 128)
            op = pp_pool.tile([128, 512], F32,
                              name=f"op{mt}_{nck}", tag="pp")
            for g in range(4):
                nc.tensor.matmul(
                    op[:],
                    ot_map[(j, g)][:, msl],
                    wo_sb[g][:, nck * 512:(nck + 1) * 512],
                    start=(g == 0), stop=(g == 3))
            osb = os_pool.tile([128, 512], BF16,
                               name=f"os{mt}_{nck}", tag="os")
            nc.vector.tensor_copy(osb[:], op[:])
            nc.sync.dma_start(
                out=part_d[mt * 128:(mt + 1) * 128,
                           nck * 512:(nck + 1) * 512],
                in_=osb[:])

        def outproj_units(j):
            units = []
            for mt in range(4 * j, 4 * j + 4):
                for nck in range(2):
                    def u(j=j, mt=mt, nck=nck):
                        emit_outproj_one(j, mt, nck)
                    units.append(u)
            return units

        # Q0/K0 first (paces on the xt/wq/wk DMA stream), then the first
        # four V tiles (enough for attention chunk j=0), then attention(g)
        # zipped with the trailing V tiles and later projections.
        for u in emit_proj_units(0):
            u()
        for mt in range(4):
            emit_vproj_unit(mt)

        def v_tail_units():
            units = []
            for mt in range(4, 16):
                def u(mt=mt):
                    emit_vproj_unit(mt)
                return_u = u
                units.append(return_u)
            return units

        zip_emit(att_iters_for_pair(0), v_tail_units() + emit_proj_units(1))
        for g in range(1, 3):
            zip_emit(att_iters_for_pair(g), emit_proj_units(g + 1))

        # Last pair: chunk j-1's out-projection is zipped between chunk j's
        # attention iterations so the PE keeps filler work while the
        # normalize chain drains; chunk 3's out-projection closes the tail.
        g = 3
        att3 = att_iters_for_pair(g)
        # per-j unit counts: t_max + 2
        ofs = 0
        for j in range(4):
            cnt = 4 * (j + 1) + 2
            zip_emit(att3[ofs:ofs + cnt],
                     outproj_units(j - 1) if j > 0 else [])
            ofs += cnt
        for u in outproj_units(3):
            u()


def _get_program():
    global _PROGRAM
    if _PROGRAM is None:
        _PROGRAM = _build_program()
    return _PROGRAM


_EXEC = None


def _get_executor():
    """Build the sharded PJRT executable once and reuse it across calls.

    Mirrors bass2jax.run_bass_via_pjrt's multi-core branch, but caches the
    jitted callable so repeat kernel() calls skip retracing/recompilation.
    Returns (fn, fn_nodonate, in_names, out_names, out_shapes, out_dtypes,
    mesh). fn takes globally concatenated inputs (n_cores*dim0, ...) plus
    donated zero output buffers, and returns concatenated outputs.
    """
    global _EXEC
    if _EXEC is None:
        import jax
        from jax.experimental.shard_map import shard_map
        from jax.sharding import Mesh, PartitionSpec

        from concourse import bass2jax

        bass2jax.install_neuronx_cc_hook()
        nc = _get_program()
        part_name = (nc.partition_id_tensor.name
                     if nc.partition_id_tensor else None)
        in_names, out_names, out_avals = [], [], []
        for alloc in nc.m.functions[0].allocations:
            if not isinstance(alloc, mybir.MemoryLocationSet):
                continue
            name = alloc.memorylocations[0].name
            if alloc.kind == "ExternalInput":
                if name != part_name:
                    in_names.append(name)
            elif alloc.kind == "ExternalOutput":
                out_names.append(name)
                out_avals.append(jax.core.ShapedArray(
                    tuple(alloc.tensor_shape), mybir.dt.np(alloc.dtype)))
        n_params = len(in_names)
        all_in = tuple(in_names) + tuple(out_names)
        if part_name is not None:
            all_in = all_in + (part_name,)

        def _body(*args):
            operands = list(args)
            if part_name is not None:
                operands.append(bass2jax.partition_id_tensor())
            outs = bass2jax._bass_exec_p.bind(
                *operands,
                out_avals=tuple(out_avals),
                in_names=all_in,
                out_names=tuple(out_names),
                lowering_input_output_aliases=(),
                sim_require_finite=True,
                sim_require_nnan=True,
                nc=nc)
            return tuple(outs)

        devices = jax.devices()[:N_CORES]
        mesh = Mesh(np.asarray(devices), ("core",))
        n_bufs = n_params + len(out_names)
        mapped = shard_map(_body, mesh=mesh,
                           in_specs=(PartitionSpec("core"),) * n_bufs,
                           out_specs=(PartitionSpec("core"),) * len(out_names),
                           check_rep=False)
        fn = jax.jit(mapped,
                     donate_argnums=tuple(range(n_params, n_bufs)),
                     keep_unused=True)
        # Non-donating twin: lets a timing loop reuse device-resident
        # argument buffers across calls (we write every element of every
        # output, so uninitialized result buffers are fine).
        fn_nodonate = jax.jit(mapped, keep_unused=True)
        out_shapes = [tuple(a.shape) for a in out_avals]
        out_dtypes = [a.dtype for a in out_avals]
        _EXEC = (fn, fn_nodonate, in_names, out_names, out_shapes,
                 out_dtypes, mesh)
    return _EXEC


def run_cores(in_maps):
    """Run the SPMD program on 8 cores via the cached executable."""
    fn, _, in_names, out_names, out_shapes, out_dtypes = _get_executor()[:6]
    concat_in = [np.concatenate([in_maps[c][n] for c in range(N_CORES)],
                                axis=0) for n in in_names]
    zeros = [np.zeros((N_CORES * s[0],) + s[1:], dt)
             for s, dt in zip(out_shapes, out_dtypes)]
    outs = fn(*concat_in, *zeros)
    res = []
    for c in range(N_CORES):
        res.append({
            n: np.asarray(outs[i]).reshape((N_CORES,) + out_shapes[i])[c]
            for i, n in enumerate(out_names)})
    return res


def make_in_maps(x, w_q, b_q, w_k, b_k, w_v, b_v, w_o, b_o):
    import ml_dtypes

    bf = ml_dtypes.bfloat16
    in_maps = []
    for c in range(N_CORES):
        b, hh = divmod(c, 2)
        cols = slice(hh * DH, (hh + 1) * DH)
        in_maps.append({
            "xt": np.ascontiguousarray(x[b].T).astype(bf),
            "wq": np.ascontiguousarray(w_q[:, cols]).astype(bf),
            "wk": np.ascontiguousarray(w_k[:, cols]).astype(bf),
            "wv": np.ascontiguousarray(w_v[:, cols]).astype(bf),
            "wo": np.ascontiguousarray(w_o[cols, :]).astype(bf),
            "bq": np.ascontiguousarray(b_q[cols]),
            "bk": np.ascontiguousarray(b_k[cols]),
        })
    return in_maps


def combine(parts, b_v, w_o, b_o):
    corr = (b_v @ w_o + b_o).astype(np.float32)
    out = np.empty((4, S, D), dtype=np.float32)
    for b in range(4):
        out[b] = (parts[2 * b].astype(np.float32)
                  + parts[2 * b + 1].astype(np.float32) + corr)
    return out


def kernel(x, w_q, b_q, w_k, b_k, w_v, b_v, w_o, b_o):
    x = np.asarray(x, dtype=np.float32)
    w_q = np.asarray(w_q, dtype=np.float32)
    b_q = np.asarray(b_q, dtype=np.float32)
    w_k = np.asarray(w_k, dtype=np.float32)
    b_k = np.asarray(b_k, dtype=np.float32)
    w_v = np.asarray(w_v, dtype=np.float32)
    b_v = np.asarray(b_v, dtype=np.float32)
    w_o = np.asarray(w_o, dtype=np.float32)
    b_o = np.asarray(b_o, dtype=np.float32)

    in_maps = make_in_maps(x, w_q, b_q, w_k, b_k, w_v, b_v, w_o, b_o)
    res = run_cores(in_maps)
    parts = [res[c]["part"] for c in range(N_CORES)]
    return combine(parts, b_v, w_o, b_o)
